# revision 37
# baseline (speedup 1.0000x reference)
"""Trainium2 Bass kernel for nn_Conv2dBN_fake_int8.

Math: the reference quantizes x and weight to int8 levels, then computes
out[b,l,o] = sum_k lut[qf[b,l,k]+128, qw[o,k]+128] with lut the exact
product table lut[i,j] = (i-128)*(j-128), so the LUT-GEMM is an integer
GEMM == a 3x3 pad-1 conv on the quantized values.  We verify the product
property of the passed lut on the host (cheap) and run the conv on the
TensorEngine in bf16 (all products/partial sums are integers < 2^24, so
fp32 PSUM accumulation is exact).  Quantize / dequant / fake-quant run on
the ACT/DVE/Pool engines; round-half-even is implemented with the
+/- 1.5*2^23 magic-number trick so it matches jnp.round bit-exactly.

Sharding: data-parallel over batch B=8 across the 8 NeuronCores (one image
per core); weights/scales replicated.

Performance structure (per core):
- Weights are shipped from the host already divided by the per-channel
  quant scale AND already in lhsT layout [k, o] (pair-stacked), so the
  device weight path is just DMA + 2 immediate-op DVE instructions - no PE
  transposes, no PSUM round trip.
- The 9 conv taps run as 3 K=128 "pair" matmuls (kh=0 paired with kh=1
  via a row-shifted copy of the quantized image in partitions 64-127)
  plus 3 K=64 single matmuls (kh=2) per 16-row output chunk.  The shifted
  copy is made with cross-partition DVE tensor_copy (partitions 0-63 ->
  64-127 with a one-padded-row source offset) - no SBUF->SBUF DMA.
- Quantize pipeline per pixel-half: ACT (scale+magic) -> DVE (clip in the
  +M domain) -> Pool (subtract magic, emit bf16), so the three engines
  pipeline and DVE keeps slack for the weight quantize and shift copies.
- Dequant per 512-pixel chunk is 3 ops, ACT -> DVE -> Pool, numerically
  exact vs the reference: d1 = acc*s2 + (b/sa + M) (the add performs the
  round-half-even), d2 = clip in the +M domain (M+-128 are exact fp32),
  d3 = (d2 - M)*sa (Sterbenz-exact subtract, one final rounding like the
  reference's out*sa).  Output chunks DMA out on two different HWDGE
  queues as soon as they are ready.
- The TileContext exit is reduced to a single-wait drain chain on SYNC
  (see _make_tc_class) - the walrus epilogue's own barrier + semaphore
  sweep does the rest of the cleanup.
"""

import numpy as np

# Problem shape (hardcoded; harness runs kernel.py standalone).
B, C, H, W = 8, 64, 32, 32
O, KH, KW = 64, 3, 3
OH, OW = 32, 32
L = OH * OW          # 1024
NT = KH * KW         # 9 taps
K = C * NT           # 576
PADW = W + 2         # 34
PADA = (H + 2) * PADW  # 1156
NCORES = 8
CHUNK = 512          # fp32 free elements per PSUM bank
RPC = CHUNK // OW    # output rows per PSUM chunk (16)
MAGIC = 12582912.0   # 1.5*2^23 -> fp32 round-to-nearest-even via add/sub
NBLK = 5             # 3 A-pair + 1 B-pair (K=128) + 1 single (K=64) blocks
WSB_COLS = NBLK * O + 2  # [w blocks | s2 | b/sa]

_nc_cache = {}


def _install_walrus_patches():
    """Cap walrus's own semaphore allocation and place the Bass kernel
    semaphores in a fixed low window.  (The walrus codegen epilogue always
    sweeps the full semaphore file behind its own barrier, so placement is
    about keeping allocation deterministic, not timing.)"""
    import concourse.bass as cb
    import concourse.bass_utils as bu

    if getattr(bu, "_lut_sem_patch", False):
        return
    bu._lut_sem_patch = True
    cb.get_kernel_semaphore_range = lambda: range(4, 53)
    orig = bu.get_walrus_args

    def patched(arch, tmpdir, *, dve_root=None):
        return orig(arch, tmpdir, dve_root=dve_root) + ["--max-sem-num=40", "--policy=2"]

    bu.get_walrus_args = patched


def _make_tc_class():
    """TileContext with a minimal kernel tail.

    - The exit drain is split into a chain of single-wait Drain
      instructions (this walrus build allows only one sync-wait command per
      instruction); the chain runs on SYNC and waits out every DMA
      semaphore.
    - The exit all-engine barriers and semaphore clears are dropped
      entirely: the walrus codegen epilogue itself barriers all engines and
      zeroes the whole semaphore file, which is exactly the cleanup a
      re-execution needs."""
    import concourse.tile as tile
    from concourse import mybir
    from concourse.vector_clock import ScopedClock

    class OverlapExitTC(tile.TileContext):
        def _drain_and_barrier(self, tick_clock, wait_clock):
            drain_inst = self.nc.sync.drain()
            wait_clock.add_sem_waits(
                drain_inst.ins, ScopedClock({None: tick_clock.global_clock})
            )
            si = drain_inst.ins.sync_info
            if si is not None and len(si.on_wait) > 1:
                waits = list(si.on_wait)
                updates = list(si.on_update)
                drain_inst.ins.sync_info = mybir.SyncInfo(
                    on_wait=waits[:1], on_update=[]
                )
                for i, w in enumerate(waits[1:]):
                    d = self.nc.sync.drain()
                    last = i == len(waits) - 2
                    d.ins.sync_info = mybir.SyncInfo(
                        on_wait=[w], on_update=updates if last else []
                    )
            assert self.sems is not None
            popped = self.nc._tile_sem_poison_stack.pop()
            assert popped is self._sem_poison

    return OverlapExitTC


def _build(sf: float, sa: float):
    import concourse.bass as bass
    import concourse.tile as tile
    from concourse import mybir

    _install_walrus_patches()

    dt = mybir.dt
    alu = mybir.AluOpType
    act = mybir.ActivationFunctionType

    nc = bass.Bass(
        "TRN2",
        debug=False,
        enable_asserts=False,
        target_bir_lowering=False,
        num_devices=NCORES,
    )

    x_d = nc.dram_tensor("x", [C, L], dt.float32, kind="ExternalInput").ap()
    wsb_d = nc.dram_tensor("wsb", [2 * C, WSB_COLS], dt.float32, kind="ExternalInput").ap()
    out_d = nc.dram_tensor("out", [O, L], dt.float32, kind="ExternalOutput").ap()

    HALF = L // 2
    ROWH = H // 2
    inv_sf = float(np.float32(1.0) / np.float32(sf))

    with _make_tc_class()(nc) as tc:
        with (
            tc.tile_pool(name="per", bufs=1) as per,
            tc.tile_pool(name="dq", bufs=3) as dq,
            tc.tile_pool(name="ps_acc", bufs=2, space="PSUM") as ps_acc,
        ):
            # ---------------- loads ----------------
            # Both HWDGE queues are packet-rate bound, so balance packets:
            #   ACT queue: x half0 (needed first), then wsb upper half
            #   SP queue:  wsb lower half (scales + lower weights), x half1
            x2 = per.tile([C, L], dt.float32)
            wsb = per.tile([2 * C, WSB_COLS], dt.float32)
            nc.scalar.dma_start(out=x2[:, 0:HALF], in_=x_d[:, 0:HALF])
            nc.sync.dma_start(out=wsb[0:C, :], in_=wsb_d[0:C, :])
            nc.scalar.dma_start(out=wsb[C : 2 * C, :], in_=wsb_d[C : 2 * C, :])
            nc.sync.dma_start(out=x2[:, HALF:L], in_=x_d[:, HALF:L])
            wT_sb = wsb[:, 0 : NBLK * O]
            s2_sb = wsb[0:O, NBLK * O : NBLK * O + 1]        # sf*sw/sa
            b2_sb = wsb[0:O, NBLK * O + 1 : NBLK * O + 2]    # bias/sa

            # ------- zero-padded quantized image, two stacked copies -------
            #   partitions 0-63:  P = padded quantized image
            #   partitions 64-127: P shifted up one padded row (for kh=1)
            qxa = per.tile([2 * C, PADA], dt.bfloat16)
            qa3 = qxa.rearrange("c (r col) -> c r col", col=PADW)
            # only the lower pad border needs zeroing (everything else read
            # is written); the upper copy inherits pads from the copy.
            nc.vector.memset(qxa[0:C, 0:PADW], 0.0)              # pad row 0
            side_pads = bass.AP(
                tensor=qxa.tensor, offset=qxa.offset + W + 1,
                ap=[qxa.ap[0], [PADW, H + 1], [1, 2]],
            )
            nc.vector.memset(side_pads, 0.0)
            nc.vector.memset(qxa[0:C, PADA - PADW : PADA], 0.0)  # pad row 33

            # ------- quantize x: ACT -> DVE -> Pool per pixel-half -------
            # t1 = x*(1/sf) + M  (the add rounds half-even to integer+M)
            # t2 = clip(t1, M-128, M+127)
            # qa = t2 - M  -> bf16 (exact: clipped ints)
            # ------- quantize w (already /sw and transposed on host) -------
            # split by partition half: each j1 op waits only its own DMA
            j1 = per.tile([2 * C, NBLK * O], dt.float32)
            for pg in (slice(0, C), slice(C, 2 * C)):
                nc.vector.tensor_scalar(
                    out=j1[pg, :], in0=wT_sb[pg, :], scalar1=MAGIC,
                    scalar2=MAGIC, op0=alu.add, op1=alu.subtract,
                )
            qwT = per.tile([2 * C, NBLK * O], dt.bfloat16)
            nc.vector.tensor_scalar(
                out=qwT, in0=j1, scalar1=-128.0, scalar2=127.0,
                op0=alu.max, op1=alu.min,
            )

            # Quantize x per pixel-half, with qa3's writer (i3) and all the
            # shifted cross-partition copies on DVE so every matmul has a
            # single uncovered producer engine (DVE); GPSIMD ucode datapath
            # ops are avoided entirely (they run ~18x slower and starve DVE
            # via SBUF contention).
            #   qxb holds the tap-(2,0)/(2,1) pair operand: lower = P,
            #   upper = P shifted left one column.
            qxb = per.tile([2 * C, PADA], dt.bfloat16)
            qb3 = qxb.rearrange("c (r col) -> c r col", col=PADW)
            t1 = per.tile([C, L], dt.float32)
            t2 = per.tile([C, L], dt.float32)
            copy_spans = [
                (0, RPC * PADW, PADW, (RPC + 1) * PADW),
                (RPC * PADW, (H + 1) * PADW, (RPC + 1) * PADW, (H + 2) * PADW),
            ]
            from concourse.tile import add_dep_helper

            copyA0_inst = None
            for h in range(2):
                px = slice(h * HALF, (h + 1) * HALF)
                nc.scalar.activation(
                    out=t1[:, px], in_=x2[:, px], func=act.Copy,
                    scale=inv_sf, bias=MAGIC,
                )
                t2_inst = nc.vector.tensor_scalar(
                    out=t2[:, px], in0=t1[:, px],
                    scalar1=MAGIC - 128.0, scalar2=MAGIC + 127.0,
                    op0=alu.max, op1=alu.min,
                )
                if h == 1 and copyA0_inst is not None:
                    # keep DVE program order i3h0 -> copyA0 -> t2h1 so the
                    # first matmul's operand (copyA0) is ready earliest; the
                    # greedy scheduler would otherwise run t2h1 first.
                    add_dep_helper(
                        t2_inst.ins, copyA0_inst.ins, sync=False,
                        reason="copyA0 before t2h1 on DVE",
                    )
                nc.vector.tensor_scalar(
                    out=qa3[0:C, 1 + h * ROWH : 1 + (h + 1) * ROWH, 1 : W + 1],
                    in0=t2[:, px].rearrange("c (r col) -> c r col", col=W),
                    scalar1=MAGIC, scalar2=None, op0=alu.subtract,
                )
                if h == 0:
                    d0, d1_, s0, s1 = copy_spans[0]
                    copyA0_inst = nc.vector.tensor_copy(
                        out=qxa[C : 2 * C, d0:d1_], in_=qxa[0:C, s0:s1]
                    )
            # after half 1: tile B first (the B-pair matmul that closes
            # chunk 0 needs it before the c1 A-pairs need copyA1)
            nc.vector.tensor_copy(
                out=qxb[0:C, 2 * PADW : PADA], in_=qxa[0:C, 2 * PADW : PADA]
            )
            nc.vector.tensor_copy(
                out=qb3[C : 2 * C, 2 : H + 2, 0:W],
                in_=qa3[0:C, 2 : H + 2, 1 : W + 1],
            )
            d0, d1_, s0, s1 = copy_spans[1]
            nc.vector.tensor_copy(
                out=qxa[C : 2 * C, d0:d1_], in_=qxa[0:C, s0:s1]
            )

            # ------- conv: 4 pair (K=128) + 1 single (K=64) matmuls/chunk --
            # per chunk: A-pairs first (earliest-ready operands), then the
            # tap8 single, then the B pair (its operand copies finish last).
            acc0 = ps_acc.tile([O, CHUNK], dt.float32, tag="acc0")
            acc1 = ps_acc.tile([O, CHUNK], dt.float32, tag="acc1")
            accs = [acc0, acc1]
            for n in range(L // CHUNK):
                r0 = n * RPC
                a = accs[n]
                for kw in range(KW):
                    nc.tensor.matmul(
                        a, qwT[:, kw * O : (kw + 1) * O],
                        qa3[:, r0 : r0 + RPC, kw : kw + OW],
                        start=(kw == 0), stop=False,
                    )
                nc.tensor.matmul(
                    a, qwT[0:C, 4 * O : 5 * O],
                    qa3[0:C, 2 + r0 : 2 + r0 + RPC, 2 : 2 + OW],
                    start=False, stop=False,
                )
                nc.tensor.matmul(
                    a, qwT[:, 3 * O : 4 * O],
                    qb3[:, 2 + r0 : 2 + r0 + RPC, 0:OW],
                    start=False, stop=True,
                )

            # early ACT touch of the dequant scalars so d1 needs only the
            # PE wait (covers the wsb DMA queue on ACT).
            act_cover = per.tile([O, 1], dt.float32)
            nc.scalar.mul(act_cover, s2_sb, 1.0)

            # ------- dequant + fake-quant: ACT d1 + 3 DVE ops per chunk ----
            # d1 = acc*s2 + b/sa (ACT, per-partition scale/bias); d2 =
            # (d1+M) max (M-128)  [the add rounds half-even]; d3 = (d2 min
            # (M+127)) - M; d4 = d3*sa.  Bit-exact vs the reference (clip
            # bounds in the +M domain are exact fp32, the -M subtract is
            # Sterbenz-exact, *sa is the reference's own final rounding).
            for n in range(L // CHUNK):
                # chunk 0 dequants whole (it has slack); chunk 1 - the
                # exit-gating one - dequants in two pixel-halves (pipelines
                # the ACT d1 with the DVE d2-d4) into one shared d4 tile,
                # then one DMA ships it (8 HWDGE DMAs total: one sem lane
                # each, so every DMA keeps a single sync wait).
                pieces = [(0, CHUNK)] if n == 0 else [
                    (0, CHUNK // 2), (CHUNK // 2, CHUNK)]
                d4 = dq.tile([O, CHUNK], dt.float32, tag="d4")
                for c0, c1 in pieces:
                    w = c1 - c0
                    d1 = dq.tile([O, CHUNK], dt.float32, tag="d1")
                    nc.scalar.activation(
                        out=d1[:, 0:w], in_=accs[n][:, c0:c1],
                        func=act.Identity, scale=s2_sb, bias=b2_sb,
                    )
                    d2 = dq.tile([O, CHUNK], dt.float32, tag="d2")
                    nc.vector.tensor_scalar(
                        out=d2[:, 0:w], in0=d1[:, 0:w],
                        scalar1=MAGIC, scalar2=MAGIC - 128.0,
                        op0=alu.add, op1=alu.max,
                    )
                    d3 = dq.tile([O, CHUNK], dt.float32, tag="d3")
                    nc.vector.tensor_scalar(
                        out=d3[:, 0:w], in0=d2[:, 0:w],
                        scalar1=MAGIC + 127.0, scalar2=MAGIC,
                        op0=alu.min, op1=alu.subtract,
                    )
                    nc.vector.tensor_scalar(
                        out=d4[:, c0:c1], in0=d3[:, 0:w],
                        scalar1=float(sa), scalar2=None, op0=alu.mult,
                    )
                if n == 0:
                    nc.sync.dma_start(
                        out=out_d[:, 0:CHUNK], in_=d4
                    )
                else:
                    # ship each finished half immediately, one per queue;
                    # each DMA waits only its own half's d4 writer
                    nc.scalar.dma_start(
                        out=out_d[:, CHUNK : CHUNK + CHUNK // 2],
                        in_=d4[:, 0 : CHUNK // 2],
                    )
                    nc.sync.dma_start(
                        out=out_d[:, CHUNK + CHUNK // 2 : L],
                        in_=d4[:, CHUNK // 2 : CHUNK],
                    )

    return nc


def _get_nc(scale_feature, scale_activation):
    sf = float(np.float32(scale_feature))
    sa = float(np.float32(scale_activation))
    key = (sf, sa)
    if key not in _nc_cache:
        _nc_cache[key] = _build(sf, sa)
    return _nc_cache[key]


def _make_in_maps(x, weight, scale_weight, bias, scale_feature, scale_activation):
    sf = np.float32(scale_feature)
    sa = np.float32(scale_activation)
    sw = scale_weight.reshape(O).astype(np.float32)
    b = bias.reshape(O).astype(np.float32)
    s2 = (sf * sw) / sa                      # fp32 per-channel dequant scale
    b2 = b / sa                              # fp32 bias in activation-steps

    wr = weight.reshape(O, C, NT).astype(np.float32)
    wdiv = wr / sw[:, None, None]            # same fp32 divide as reference
    wsb = np.zeros((2 * C, WSB_COLS), dtype=np.float32)
    for kw in range(KW):                     # A-pair blocks: kh=0 | kh=1
        wsb[0:C, kw * O : (kw + 1) * O] = wdiv[:, :, kw].T
        wsb[C : 2 * C, kw * O : (kw + 1) * O] = wdiv[:, :, KW + kw].T
    wsb[0:C, 3 * O : 4 * O] = wdiv[:, :, 6].T       # B pair: (2,0) | (2,1)
    wsb[C : 2 * C, 3 * O : 4 * O] = wdiv[:, :, 7].T
    wsb[0:C, 4 * O : 5 * O] = wdiv[:, :, 8].T       # single: (2,2)
    wsb[0:O, NBLK * O] = s2
    wsb[0:O, NBLK * O + 1] = b2
    wsb = np.ascontiguousarray(wsb)
    return [
        {
            "x": np.ascontiguousarray(x[bb].reshape(C, L), dtype=np.float32),
            "wsb": wsb,
        }
        for bb in range(B)
    ]


def _kernel_device(x, weight, scale_feature, scale_weight, scale_activation, bias):
    from concourse import bass_utils

    nc = _get_nc(scale_feature, scale_activation)
    in_maps = _make_in_maps(
        x, weight, scale_weight, bias, scale_feature, scale_activation
    )
    res = bass_utils.run_bass_kernel_spmd(nc, in_maps, core_ids=list(range(NCORES)))
    return np.stack([r["out"].reshape(O, OH, OW) for r in res.results]).astype(
        np.float32
    )


def _kernel_numpy_lut(x, weight, lut, sf, sw, sa, bias):
    """Honest LUT-GEMM fallback (only if lut is not the product table)."""
    qf = np.clip(np.round(x / np.float32(sf)), -128.0, 127.0)
    qw = np.clip(np.round(weight / sw[:, None, None, None]), -128.0, 127.0)
    idx_w = qw.reshape(O, K).astype(np.int64) + 128
    qfp = np.pad(qf, ((0, 0), (0, 0), (1, 1), (1, 1)))
    acc = np.zeros((B, L, O), np.int64)
    for t in range(NT):
        kh, kw = divmod(t, KW)
        win = qfp[:, :, kh : kh + OH, kw : kw + OW].reshape(B, C, L)
        idx_f = win.astype(np.int64) + 128  # [B, C, L]
        for c in range(C):
            acc += lut[idx_f[:, c, :, None], idx_w[None, None, :, c * NT + t]]
    out = acc.astype(np.float32).transpose(0, 2, 1).reshape(B, O, OH, OW)
    out = out * np.float32(sf) * sw[None, :, None, None]
    out = out + bias[None, :, None, None]
    out = np.round(out / np.float32(sa))
    out = np.clip(out, -128.0, 127.0)
    return (out * np.float32(sa)).astype(np.float32)


def kernel(x, weight, lut, scale_feature, scale_weight, scale_activation, bias):
    x = np.asarray(x, dtype=np.float32)
    weight = np.asarray(weight, dtype=np.float32)
    lut = np.asarray(lut)
    scale_weight = np.asarray(scale_weight, dtype=np.float32)
    bias = np.asarray(bias, dtype=np.float32)

    i = np.arange(256, dtype=np.int64) - 128
    product = i[:, None] * i[None, :]
    if not np.array_equal(np.asarray(lut, dtype=np.int64), product):
        return _kernel_numpy_lut(
            x, weight, np.asarray(lut, dtype=np.int64),
            float(np.float32(scale_feature)), scale_weight,
            float(np.float32(scale_activation)), bias,
        )

    return _kernel_device(
        x, weight, scale_feature, scale_weight, scale_activation, bias
    )


# revision 38
# speedup vs baseline: 1.0099x; 1.0099x over previous
"""Trainium2 Bass kernel for nn_Conv2dBN_fake_int8.

Math: the reference quantizes x and weight to int8 levels, then computes
out[b,l,o] = sum_k lut[qf[b,l,k]+128, qw[o,k]+128] with lut the exact
product table lut[i,j] = (i-128)*(j-128), so the LUT-GEMM is an integer
GEMM == a 3x3 pad-1 conv on the quantized values.  We verify the product
property of the passed lut on the host (cheap) and run the conv on the
TensorEngine in bf16 (all products/partial sums are integers < 2^24, so
fp32 PSUM accumulation is exact).  Quantize / dequant / fake-quant run on
the ACT/DVE/Pool engines; round-half-even is implemented with the
+/- 1.5*2^23 magic-number trick so it matches jnp.round bit-exactly.

Sharding: data-parallel over batch B=8 across the 8 NeuronCores (one image
per core); weights/scales replicated.

Performance structure (per core):
- Weights are shipped from the host already divided by the per-channel
  quant scale AND already in lhsT layout [k, o] (pair-stacked), so the
  device weight path is just DMA + 2 immediate-op DVE instructions - no PE
  transposes, no PSUM round trip.
- The 9 conv taps run as 3 K=128 "pair" matmuls (kh=0 paired with kh=1
  via a row-shifted copy of the quantized image in partitions 64-127)
  plus 3 K=64 single matmuls (kh=2) per 16-row output chunk.  The shifted
  copy is made with cross-partition DVE tensor_copy (partitions 0-63 ->
  64-127 with a one-padded-row source offset) - no SBUF->SBUF DMA.
- Quantize pipeline per pixel-half: ACT (scale+magic) -> DVE (clip in the
  +M domain) -> Pool (subtract magic, emit bf16), so the three engines
  pipeline and DVE keeps slack for the weight quantize and shift copies.
- Dequant per 512-pixel chunk is 3 ops, ACT -> DVE -> Pool, numerically
  exact vs the reference: d1 = acc*s2 + (b/sa + M) (the add performs the
  round-half-even), d2 = clip in the +M domain (M+-128 are exact fp32),
  d3 = (d2 - M)*sa (Sterbenz-exact subtract, one final rounding like the
  reference's out*sa).  Output chunks DMA out on two different HWDGE
  queues as soon as they are ready.
- The TileContext exit is reduced to a single-wait drain chain on SYNC
  (see _make_tc_class) - the walrus epilogue's own barrier + semaphore
  sweep does the rest of the cleanup.
"""

import numpy as np

# Problem shape (hardcoded; harness runs kernel.py standalone).
B, C, H, W = 8, 64, 32, 32
O, KH, KW = 64, 3, 3
OH, OW = 32, 32
L = OH * OW          # 1024
NT = KH * KW         # 9 taps
K = C * NT           # 576
PADW = W + 2         # 34
PADA = (H + 2) * PADW  # 1156
NCORES = 8
CHUNK = 512          # fp32 free elements per PSUM bank
RPC = CHUNK // OW    # output rows per PSUM chunk (16)
MAGIC = 12582912.0   # 1.5*2^23 -> fp32 round-to-nearest-even via add/sub
NBLK = 5             # 3 A-pair + 1 B-pair (K=128) + 1 single (K=64) blocks
WSB_COLS = NBLK * O + 2  # [w blocks | s2 | b/sa]

_nc_cache = {}


def _install_walrus_patches():
    """Cap walrus's own semaphore allocation and place the Bass kernel
    semaphores in a fixed low window.  (The walrus codegen epilogue always
    sweeps the full semaphore file behind its own barrier, so placement is
    about keeping allocation deterministic, not timing.)"""
    import concourse.bass as cb
    import concourse.bass_utils as bu

    if getattr(bu, "_lut_sem_patch", False):
        return
    bu._lut_sem_patch = True
    cb.get_kernel_semaphore_range = lambda: range(4, 53)
    orig = bu.get_walrus_args

    def patched(arch, tmpdir, *, dve_root=None):
        return orig(arch, tmpdir, dve_root=dve_root) + ["--max-sem-num=40"]

    bu.get_walrus_args = patched


def _make_tc_class():
    """TileContext with a minimal kernel tail.

    - The exit drain is split into a chain of single-wait Drain
      instructions (this walrus build allows only one sync-wait command per
      instruction); the chain runs on SYNC and waits out every DMA
      semaphore.
    - The exit all-engine barriers and semaphore clears are dropped
      entirely: the walrus codegen epilogue itself barriers all engines and
      zeroes the whole semaphore file, which is exactly the cleanup a
      re-execution needs."""
    import concourse.tile as tile
    from concourse import mybir
    from concourse.vector_clock import ScopedClock

    class OverlapExitTC(tile.TileContext):
        def _drain_and_barrier(self, tick_clock, wait_clock):
            drain_inst = self.nc.sync.drain()
            wait_clock.add_sem_waits(
                drain_inst.ins, ScopedClock({None: tick_clock.global_clock})
            )
            si = drain_inst.ins.sync_info
            if si is not None and len(si.on_wait) > 1:
                waits = list(si.on_wait)
                updates = list(si.on_update)
                drain_inst.ins.sync_info = mybir.SyncInfo(
                    on_wait=waits[:1], on_update=[]
                )
                for i, w in enumerate(waits[1:]):
                    d = self.nc.sync.drain()
                    last = i == len(waits) - 2
                    d.ins.sync_info = mybir.SyncInfo(
                        on_wait=[w], on_update=updates if last else []
                    )
            assert self.sems is not None
            popped = self.nc._tile_sem_poison_stack.pop()
            assert popped is self._sem_poison

    return OverlapExitTC


def _build(sf: float, sa: float):
    import concourse.bass as bass
    import concourse.tile as tile
    from concourse import mybir

    _install_walrus_patches()

    dt = mybir.dt
    alu = mybir.AluOpType
    act = mybir.ActivationFunctionType

    nc = bass.Bass(
        "TRN2",
        debug=False,
        enable_asserts=False,
        target_bir_lowering=False,
        num_devices=NCORES,
    )

    x_d = nc.dram_tensor("x", [C, L], dt.float32, kind="ExternalInput").ap()
    wsb_d = nc.dram_tensor("wsb", [2 * C, WSB_COLS], dt.float32, kind="ExternalInput").ap()
    out_d = nc.dram_tensor("out", [O, L], dt.float32, kind="ExternalOutput").ap()

    HALF = L // 2
    ROWH = H // 2
    inv_sf = float(np.float32(1.0) / np.float32(sf))

    with _make_tc_class()(nc) as tc:
        with (
            tc.tile_pool(name="per", bufs=1) as per,
            tc.tile_pool(name="dq", bufs=3) as dq,
            tc.tile_pool(name="ps_acc", bufs=2, space="PSUM") as ps_acc,
        ):
            # ---------------- loads ----------------
            # Both HWDGE queues are packet-rate bound, so balance packets:
            #   ACT queue: x half0 (needed first), then wsb upper half
            #   SP queue:  wsb lower half (scales + lower weights), x half1
            x2 = per.tile([C, L], dt.float32)
            wsb = per.tile([2 * C, WSB_COLS], dt.float32)
            nc.scalar.dma_start(out=x2[:, 0:HALF], in_=x_d[:, 0:HALF])
            nc.sync.dma_start(out=wsb[0:C, :], in_=wsb_d[0:C, :])
            nc.scalar.dma_start(out=wsb[C : 2 * C, :], in_=wsb_d[C : 2 * C, :])
            nc.sync.dma_start(out=x2[:, HALF:L], in_=x_d[:, HALF:L])
            wT_sb = wsb[:, 0 : NBLK * O]
            s2_sb = wsb[0:O, NBLK * O : NBLK * O + 1]        # sf*sw/sa
            b2_sb = wsb[0:O, NBLK * O + 1 : NBLK * O + 2]    # bias/sa

            # ------- zero-padded quantized image, two stacked copies -------
            #   partitions 0-63:  P = padded quantized image
            #   partitions 64-127: P shifted up one padded row (for kh=1)
            qxa = per.tile([2 * C, PADA], dt.bfloat16)
            qa3 = qxa.rearrange("c (r col) -> c r col", col=PADW)
            # only the lower pad border needs zeroing (everything else read
            # is written); the upper copy inherits pads from the copy.
            nc.vector.memset(qxa[0:C, 0:PADW], 0.0)              # pad row 0
            side_pads = bass.AP(
                tensor=qxa.tensor, offset=qxa.offset + W + 1,
                ap=[qxa.ap[0], [PADW, H + 1], [1, 2]],
            )
            nc.vector.memset(side_pads, 0.0)
            nc.vector.memset(qxa[0:C, PADA - PADW : PADA], 0.0)  # pad row 33

            # ------- quantize x: ACT -> DVE -> Pool per pixel-half -------
            # t1 = x*(1/sf) + M  (the add rounds half-even to integer+M)
            # t2 = clip(t1, M-128, M+127)
            # qa = t2 - M  -> bf16 (exact: clipped ints)
            # ------- quantize w (already /sw and transposed on host) -------
            # split by partition half: each j1 op waits only its own DMA
            j1 = per.tile([2 * C, NBLK * O], dt.float32)
            for pg in (slice(0, C), slice(C, 2 * C)):
                nc.vector.tensor_scalar(
                    out=j1[pg, :], in0=wT_sb[pg, :], scalar1=MAGIC,
                    scalar2=MAGIC, op0=alu.add, op1=alu.subtract,
                )
            qwT = per.tile([2 * C, NBLK * O], dt.bfloat16)
            nc.vector.tensor_scalar(
                out=qwT, in0=j1, scalar1=-128.0, scalar2=127.0,
                op0=alu.max, op1=alu.min,
            )

            # Quantize x per pixel-half, with qa3's writer (i3) and all the
            # shifted cross-partition copies on DVE so every matmul has a
            # single uncovered producer engine (DVE); GPSIMD ucode datapath
            # ops are avoided entirely (they run ~18x slower and starve DVE
            # via SBUF contention).
            #   qxb holds the tap-(2,0)/(2,1) pair operand: lower = P,
            #   upper = P shifted left one column.
            qxb = per.tile([2 * C, PADA], dt.bfloat16)
            qb3 = qxb.rearrange("c (r col) -> c r col", col=PADW)
            t1 = per.tile([C, L], dt.float32)
            t2 = per.tile([C, L], dt.float32)
            copy_spans = [
                (0, RPC * PADW, PADW, (RPC + 1) * PADW),
                (RPC * PADW, (H + 1) * PADW, (RPC + 1) * PADW, (H + 2) * PADW),
            ]
            from concourse.tile import add_dep_helper

            copyA0_inst = None
            for h in range(2):
                px = slice(h * HALF, (h + 1) * HALF)
                nc.scalar.activation(
                    out=t1[:, px], in_=x2[:, px], func=act.Copy,
                    scale=inv_sf, bias=MAGIC,
                )
                t2_inst = nc.vector.tensor_scalar(
                    out=t2[:, px], in0=t1[:, px],
                    scalar1=MAGIC - 128.0, scalar2=MAGIC + 127.0,
                    op0=alu.max, op1=alu.min,
                )
                if h == 1 and copyA0_inst is not None:
                    # keep DVE program order i3h0 -> copyA0 -> t2h1 so the
                    # first matmul's operand (copyA0) is ready earliest; the
                    # greedy scheduler would otherwise run t2h1 first.
                    add_dep_helper(
                        t2_inst.ins, copyA0_inst.ins, sync=False,
                        reason="copyA0 before t2h1 on DVE",
                    )
                nc.vector.tensor_scalar(
                    out=qa3[0:C, 1 + h * ROWH : 1 + (h + 1) * ROWH, 1 : W + 1],
                    in0=t2[:, px].rearrange("c (r col) -> c r col", col=W),
                    scalar1=MAGIC, scalar2=None, op0=alu.subtract,
                )
                if h == 0:
                    d0, d1_, s0, s1 = copy_spans[0]
                    copyA0_inst = nc.vector.tensor_copy(
                        out=qxa[C : 2 * C, d0:d1_], in_=qxa[0:C, s0:s1]
                    )
            # after half 1: tile B first (the B-pair matmul that closes
            # chunk 0 needs it before the c1 A-pairs need copyA1)
            nc.vector.tensor_copy(
                out=qxb[0:C, 2 * PADW : PADA], in_=qxa[0:C, 2 * PADW : PADA]
            )
            nc.vector.tensor_copy(
                out=qb3[C : 2 * C, 2 : H + 2, 0:W],
                in_=qa3[0:C, 2 : H + 2, 1 : W + 1],
            )
            d0, d1_, s0, s1 = copy_spans[1]
            nc.vector.tensor_copy(
                out=qxa[C : 2 * C, d0:d1_], in_=qxa[0:C, s0:s1]
            )

            # ------- conv: 4 pair (K=128) + 1 single (K=64) matmuls/chunk --
            # per chunk: A-pairs first (earliest-ready operands), then the
            # tap8 single, then the B pair (its operand copies finish last).
            acc0 = ps_acc.tile([O, CHUNK], dt.float32, tag="acc0")
            acc1 = ps_acc.tile([O, CHUNK], dt.float32, tag="acc1")
            accs = [acc0, acc1]
            for n in range(L // CHUNK):
                r0 = n * RPC
                a = accs[n]
                for kw in range(KW):
                    nc.tensor.matmul(
                        a, qwT[:, kw * O : (kw + 1) * O],
                        qa3[:, r0 : r0 + RPC, kw : kw + OW],
                        start=(kw == 0), stop=False,
                    )
                nc.tensor.matmul(
                    a, qwT[0:C, 4 * O : 5 * O],
                    qa3[0:C, 2 + r0 : 2 + r0 + RPC, 2 : 2 + OW],
                    start=False, stop=False,
                )
                nc.tensor.matmul(
                    a, qwT[:, 3 * O : 4 * O],
                    qb3[:, 2 + r0 : 2 + r0 + RPC, 0:OW],
                    start=False, stop=True,
                )

            # early ACT touch of the dequant scalars so d1 needs only the
            # PE wait (covers the wsb DMA queue on ACT).
            act_cover = per.tile([O, 1], dt.float32)
            nc.scalar.mul(act_cover, s2_sb, 1.0)

            # ------- dequant + fake-quant: ACT d1 + 3 DVE ops per chunk ----
            # d1 = acc*s2 + b/sa (ACT, per-partition scale/bias); d2 =
            # (d1+M) max (M-128)  [the add rounds half-even]; d3 = (d2 min
            # (M+127)) - M; d4 = d3*sa.  Bit-exact vs the reference (clip
            # bounds in the +M domain are exact fp32, the -M subtract is
            # Sterbenz-exact, *sa is the reference's own final rounding).
            for n in range(L // CHUNK):
                # chunk 0 dequants whole (it has slack); chunk 1 - the
                # exit-gating one - dequants in two pixel-halves (pipelines
                # the ACT d1 with the DVE d2-d4) into one shared d4 tile,
                # then one DMA ships it (8 HWDGE DMAs total: one sem lane
                # each, so every DMA keeps a single sync wait).
                pieces = [(0, CHUNK)] if n == 0 else [
                    (0, CHUNK // 2), (CHUNK // 2, CHUNK)]
                d4 = dq.tile([O, CHUNK], dt.float32, tag="d4")
                for c0, c1 in pieces:
                    w = c1 - c0
                    d1 = dq.tile([O, CHUNK], dt.float32, tag="d1")
                    nc.scalar.activation(
                        out=d1[:, 0:w], in_=accs[n][:, c0:c1],
                        func=act.Identity, scale=s2_sb, bias=b2_sb,
                    )
                    d2 = dq.tile([O, CHUNK], dt.float32, tag="d2")
                    nc.vector.tensor_scalar(
                        out=d2[:, 0:w], in0=d1[:, 0:w],
                        scalar1=MAGIC, scalar2=MAGIC - 128.0,
                        op0=alu.add, op1=alu.max,
                    )
                    d3 = dq.tile([O, CHUNK], dt.float32, tag="d3")
                    nc.vector.tensor_scalar(
                        out=d3[:, 0:w], in0=d2[:, 0:w],
                        scalar1=MAGIC + 127.0, scalar2=MAGIC,
                        op0=alu.min, op1=alu.subtract,
                    )
                    nc.vector.tensor_scalar(
                        out=d4[:, c0:c1], in0=d3[:, 0:w],
                        scalar1=float(sa), scalar2=None, op0=alu.mult,
                    )
                if n == 0:
                    # split across both queues so each is clear again before
                    # the exit-gating chunk-1 DMAs arrive
                    nc.sync.dma_start(
                        out=out_d[0 : O // 2, 0:CHUNK], in_=d4[0 : O // 2, :]
                    )
                    nc.scalar.dma_start(
                        out=out_d[O // 2 : O, 0:CHUNK], in_=d4[O // 2 : O, :]
                    )
                else:
                    # ship each finished half immediately, one per queue;
                    # each DMA waits only its own half's d4 writer
                    nc.scalar.dma_start(
                        out=out_d[:, CHUNK : CHUNK + CHUNK // 2],
                        in_=d4[:, 0 : CHUNK // 2],
                    )
                    nc.sync.dma_start(
                        out=out_d[:, CHUNK + CHUNK // 2 : L],
                        in_=d4[:, CHUNK // 2 : CHUNK],
                    )

    return nc


def _get_nc(scale_feature, scale_activation):
    sf = float(np.float32(scale_feature))
    sa = float(np.float32(scale_activation))
    key = (sf, sa)
    if key not in _nc_cache:
        _nc_cache[key] = _build(sf, sa)
    return _nc_cache[key]


def _make_in_maps(x, weight, scale_weight, bias, scale_feature, scale_activation):
    sf = np.float32(scale_feature)
    sa = np.float32(scale_activation)
    sw = scale_weight.reshape(O).astype(np.float32)
    b = bias.reshape(O).astype(np.float32)
    s2 = (sf * sw) / sa                      # fp32 per-channel dequant scale
    b2 = b / sa                              # fp32 bias in activation-steps

    wr = weight.reshape(O, C, NT).astype(np.float32)
    wdiv = wr / sw[:, None, None]            # same fp32 divide as reference
    wsb = np.zeros((2 * C, WSB_COLS), dtype=np.float32)
    for kw in range(KW):                     # A-pair blocks: kh=0 | kh=1
        wsb[0:C, kw * O : (kw + 1) * O] = wdiv[:, :, kw].T
        wsb[C : 2 * C, kw * O : (kw + 1) * O] = wdiv[:, :, KW + kw].T
    wsb[0:C, 3 * O : 4 * O] = wdiv[:, :, 6].T       # B pair: (2,0) | (2,1)
    wsb[C : 2 * C, 3 * O : 4 * O] = wdiv[:, :, 7].T
    wsb[0:C, 4 * O : 5 * O] = wdiv[:, :, 8].T       # single: (2,2)
    wsb[0:O, NBLK * O] = s2
    wsb[0:O, NBLK * O + 1] = b2
    wsb = np.ascontiguousarray(wsb)
    return [
        {
            "x": np.ascontiguousarray(x[bb].reshape(C, L), dtype=np.float32),
            "wsb": wsb,
        }
        for bb in range(B)
    ]


def _kernel_device(x, weight, scale_feature, scale_weight, scale_activation, bias):
    from concourse import bass_utils

    nc = _get_nc(scale_feature, scale_activation)
    in_maps = _make_in_maps(
        x, weight, scale_weight, bias, scale_feature, scale_activation
    )
    res = bass_utils.run_bass_kernel_spmd(nc, in_maps, core_ids=list(range(NCORES)))
    return np.stack([r["out"].reshape(O, OH, OW) for r in res.results]).astype(
        np.float32
    )


def _kernel_numpy_lut(x, weight, lut, sf, sw, sa, bias):
    """Honest LUT-GEMM fallback (only if lut is not the product table)."""
    qf = np.clip(np.round(x / np.float32(sf)), -128.0, 127.0)
    qw = np.clip(np.round(weight / sw[:, None, None, None]), -128.0, 127.0)
    idx_w = qw.reshape(O, K).astype(np.int64) + 128
    qfp = np.pad(qf, ((0, 0), (0, 0), (1, 1), (1, 1)))
    acc = np.zeros((B, L, O), np.int64)
    for t in range(NT):
        kh, kw = divmod(t, KW)
        win = qfp[:, :, kh : kh + OH, kw : kw + OW].reshape(B, C, L)
        idx_f = win.astype(np.int64) + 128  # [B, C, L]
        for c in range(C):
            acc += lut[idx_f[:, c, :, None], idx_w[None, None, :, c * NT + t]]
    out = acc.astype(np.float32).transpose(0, 2, 1).reshape(B, O, OH, OW)
    out = out * np.float32(sf) * sw[None, :, None, None]
    out = out + bias[None, :, None, None]
    out = np.round(out / np.float32(sa))
    out = np.clip(out, -128.0, 127.0)
    return (out * np.float32(sa)).astype(np.float32)


def kernel(x, weight, lut, scale_feature, scale_weight, scale_activation, bias):
    x = np.asarray(x, dtype=np.float32)
    weight = np.asarray(weight, dtype=np.float32)
    lut = np.asarray(lut)
    scale_weight = np.asarray(scale_weight, dtype=np.float32)
    bias = np.asarray(bias, dtype=np.float32)

    i = np.arange(256, dtype=np.int64) - 128
    product = i[:, None] * i[None, :]
    if not np.array_equal(np.asarray(lut, dtype=np.int64), product):
        return _kernel_numpy_lut(
            x, weight, np.asarray(lut, dtype=np.int64),
            float(np.float32(scale_feature)), scale_weight,
            float(np.float32(scale_activation)), bias,
        )

    return _kernel_device(
        x, weight, scale_feature, scale_weight, scale_activation, bias
    )
